# revision 11
# baseline (speedup 1.0000x reference)
"""Trainium2 Bass kernel for nn_Decoder_31044023616500.

Pointer-network decoder: 2-layer LSTM over N=640 steps, an N x N x L tanh
pointer-score grid, and a label MLP.  Runs SPMD on 8 NeuronCores; the grid
and label MLP are sharded row-wise (80 rows/core), the sequential LSTM is
replicated on every core.

Precision scheme (validated against the fp32 reference in numpy):
  - LSTM recurrent weights / hidden history: fp16 (full-rate matmul + FWL)
  - gate pre-activations, cell state: fp32
  - big grid matmuls: float32r (full-rate fp32 on the PE)
  => scores abs err ~9e-4, labels ~1e-3, argmax exact.
"""

import sys

sys.path.insert(0, "/opt/trn_rl_repo")

import numpy as np

import concourse.bass as bass
import concourse.bacc as bacc
import concourse.tile as tile
from concourse import mybir
from concourse import bass_utils

DT = mybir.dt
F32 = DT.float32
F32R = DT.float32r
F16 = DT.float16
AF = mybir.ActivationFunctionType

N = 640          # sequence length
H = 256          # lstm width
L = 640          # pointer-MLP layer size
NCORES = 8
RPC = N // NCORES  # rows per core = 80
BLK = 64           # LSTM pass-interleave block
NBLK = N // BLK

# permuted gate chunk order: [i0 i1 f0 f1 o0 o1 g0 g1] (orig order i,f,g,o)
GATE_PERM = [0, 1, 2, 3, 6, 7, 4, 5]


def _perm_rows(w):
    """Permute dim-0 (4H) gate chunks of w into GATE_PERM order."""
    chunks = w.reshape(8, 128, *w.shape[1:])
    return np.concatenate([chunks[c] for c in GATE_PERM], axis=0)


def _pack_lhsT(wt, n_k, n_m, dtype):
    """wt: [K, M] (already transposed weight).  Returns [128, n_k*n_m*128]
    with tile (kc, mc) at column block kc*n_m + mc."""
    K, M = wt.shape
    assert K == n_k * 128 and M == n_m * 128
    out = np.empty((128, n_k * n_m * 128), dtype)
    for kc in range(n_k):
        for mc in range(n_m):
            ti = kc * n_m + mc
            out[:, ti * 128:(ti + 1) * 128] = wt[kc * 128:(kc + 1) * 128,
                                                 mc * 128:(mc + 1) * 128]
    return np.ascontiguousarray(out)


def _f(a, dtype=np.float32):
    return np.ascontiguousarray(np.asarray(a, dtype=dtype))


def _split_drain_waits(nc):
    """This walrus build only supports ONE sync-wait on CTRL-class (Drain)
    instructions; Tile's kernel-tail drain carries one wait per active proc.
    Hoist excess waits onto inserted single-wait drains."""
    fn = nc.m.functions[0]
    for b in fn.blocks:
        insts = list(b.instructions)
        out = []
        changed = False
        for inst in insts:
            si = inst.sync_info
            if inst.opcode == "Drain" and si and si.on_wait and \
                    len(si.on_wait) > 1:
                changed = True
                waits = list(si.on_wait)
                for w in waits[:-1]:
                    d = mybir.InstDrain(
                        name=nc.get_next_instruction_name(),
                        ins=[], outs=[], bass_is_fusable=False)
                    d.engine = inst.engine
                    d.sync_info = mybir.SyncInfo(on_wait=[w], on_update=[])
                    out.append(d)
                si.on_wait = waits[-1:]
            out.append(inst)
        if changed:
            b.instructions = out


def build_program(by2_val: float):
    """Builds the Bass program.  Returns nc."""
    nc = bacc.Bacc("TRN2", target_bir_lowering=False, debug=False,
                   num_devices=NCORES)

    def din(name, shape, dt):
        return nc.dram_tensor(name, shape, dt, kind="ExternalInput").ap()

    def dout(name, shape, dt):
        return nc.dram_tensor(name, shape, dt, kind="ExternalOutput").ap()

    # ---- DRAM inputs ----
    d_whh0t = din("whh0t", [128, 16 * 128], F16)
    d_whh1t = din("whh1t", [128, 16 * 128], F16)
    d_wih1t = din("wih1t", [128, 16 * 128], F16)
    d_wih0t = din("wih0t", [128, 32 * 128], F32)
    d_b0 = din("b0", [128, 8], F32)
    d_b1 = din("b1", [128, 8], F32)
    d_xt = din("xt", [4, 128, N], F32)
    d_wy1at = din("wy1at", [128, 10 * 128], F16)
    d_wy1bt = din("wy1bt", [128, 20 * 128], F32)
    d_wy1ct = din("wy1ct", [128, 20 * 128], F32)
    d_by1c = din("by1c", [128, 5], F32)
    d_wy2c = din("wy2c", [128, 5], F16)
    d_wxct = din("wxct", [128, 40 * 128], F32)
    d_wzdect = din("wzdect", [128, 10 * 128], F16)
    d_bz1c = din("bz1c", [128, 5], F32)
    d_wz2t = din("wz2t", [128, 5 * 36], F32)
    d_bz2c = din("bz2c", [36, 1], F32)
    d_ident = din("identf", [128, 128], F32)
    d_xxtsel = din("xxtsel", [8, 128, RPC], F32)   # per-core

    # ---- DRAM outputs (per core) ----
    d_scores = dout("scores_part", [RPC, N], F32)
    d_labelsT = dout("labelsT_part", [36, RPC], F32)
    d_lidx = dout("lidx_part", [RPC, 1], DT.int32)
    d_dec = dout("dec_out", [128, 2, N], F16)

    with tile.TileContext(nc) as tc:
        import contextlib
        ctx = contextlib.ExitStack()
        with ctx:
            consts = ctx.enter_context(tc.tile_pool(name="consts", bufs=1))

            def cload(dram, shape, dt, tag):
                t = consts.tile(shape, dt, tag=tag)
                nc.sync.dma_start(t, dram)
                return t

            whh0t = cload(d_whh0t, [128, 2048], F16, "whh0t")
            whh1t = cload(d_whh1t, [128, 2048], F16, "whh1t")
            wih1t = cload(d_wih1t, [128, 2048], F16, "wih1t")
            wih0t = cload(d_wih0t, [128, 4096], F32, "wih0t")
            b0 = cload(d_b0, [128, 8], F32, "b0")
            b1 = cload(d_b1, [128, 8], F32, "b1")
            xt = consts.tile([128, 4, N], F32, tag="xt")
            for kc in range(4):
                nc.sync.dma_start(xt[:, kc, :], d_xt[kc, :, :])
            wy1at = cload(d_wy1at, [128, 1280], F16, "wy1at")
            wy1bt = cload(d_wy1bt, [128, 2560], F32, "wy1bt")
            wy1ct = cload(d_wy1ct, [128, 2560], F32, "wy1ct")
            by1c = cload(d_by1c, [128, 5], F32, "by1c")
            wy2c = cload(d_wy2c, [128, 5], F16, "wy2c")
            wxct = cload(d_wxct, [128, 5120], F32, "wxct")
            wzdect = cload(d_wzdect, [128, 1280], F16, "wzdect")
            bz1c = cload(d_bz1c, [128, 5], F32, "bz1c")
            wz2t = cload(d_wz2t, [128, 180], F32, "wz2t")
            bz2c = cload(d_bz2c, [36, 1], F32, "bz2c")
            ident = cload(d_ident, [128, 128], F32, "identf")
            xxtsel = consts.tile([128, 8, RPC], F32, tag="xxtsel")
            for kc in range(8):
                nc.sync.dma_start(xxtsel[:, kc, :], d_xxtsel[kc, :, :])

            hist0 = consts.tile([128, 2, N + 1], F16, tag="hist0")
            hist1 = consts.tile([128, 2, N + 1], F16, tag="hist1")
            c0 = consts.tile([128, 2], F32, tag="c0")
            c1 = consts.tile([128, 2], F32, tag="c1")
            pre0 = consts.tile([128, 8, N], F32, tag="pre0")
            pre1 = consts.tile([128, 8, N], F32, tag="pre1")
            pT = consts.tile([128, 5, N], F32, tag="pT")
            qT = consts.tile([128, 5, N], F32, tag="qT")
            pb = consts.tile([128, 5, RPC], F32, tag="pb")
            decsel = consts.tile([128, 2, RPC], F16, tag="decsel")
            hz = consts.tile([128, 5, RPC], F32, tag="hz")

            nc.vector.memset(hist0[:, :, 0], 0.0)
            nc.vector.memset(hist1[:, :, 0], 0.0)
            nc.vector.memset(c0, 0.0)
            nc.vector.memset(c1, 0.0)

            lstm_ctx = ctx.enter_context(contextlib.ExitStack())
            mmps = lstm_ctx.enter_context(
                tc.tile_pool(name="mmps", bufs=2, space="PSUM"))
            gps = lstm_ctx.enter_context(
                tc.tile_pool(name="gps", bufs=3, space="PSUM"))
            lsb = lstm_ctx.enter_context(tc.tile_pool(name="lsb", bufs=3))

            # ---- pre0 = Wih0 @ x.T + b0   (f32r) ----
            for mc in range(8):
                for n0, nsz in ((0, 512), (512, 128)):
                    ps = mmps.tile([128, 512], F32, tag="mm")
                    for kc in range(4):
                        ti = kc * 8 + mc
                        nc.tensor.matmul(
                            ps[:, :nsz],
                            wih0t[:, ti * 128:(ti + 1) * 128],
                            xt[:, kc, n0:n0 + nsz],
                            start=(kc == 0), stop=(kc == 3))
                    nc.vector.tensor_scalar_add(
                        pre0[:, mc, n0:n0 + nsz], ps[:, :nsz],
                        b0[:, mc:mc + 1])

            # ---- Q^T = Wy1c @ x.T + by1   (f32r) ----
            for lc in range(5):
                for jh in range(2):
                    ps = mmps.tile([128, 512], F32, tag="mm")
                    for kc in range(4):
                        ti = kc * 5 + lc
                        nc.tensor.matmul(
                            ps[:, :320],
                            wy1ct[:, ti * 128:(ti + 1) * 128],
                            xt[:, kc, jh * 320:(jh + 1) * 320],
                            start=(kc == 0), stop=(kc == 3))
                    nc.vector.tensor_scalar_add(
                        qT[:, lc, jh * 320:(jh + 1) * 320], ps[:, :320],
                        by1c[:, lc:lc + 1])

            # ---- LSTM ----
            def lstm_step(layer, t):
                hist, c_sb, pre, wt = ((hist0, c0, pre0, whh0t) if layer == 0
                                       else (hist1, c1, pre1, whh1t))
                g_ps = gps.tile([128, 8], F32, tag="g")
                for mc in range(8):
                    for kc in range(2):
                        ti = kc * 8 + mc
                        nc.tensor.matmul(
                            g_ps[:, mc:mc + 1],
                            wt[:, ti * 128:(ti + 1) * 128],
                            hist[:, kc, t:t + 1],
                            start=(kc == 0), stop=(kc == 1))
                g_sb = lsb.tile([128, 8], F32, tag="gsb")
                nc.vector.tensor_add(g_sb, g_ps, pre[:, :, t])
                s_sb = lsb.tile([128, 6], F32, tag="s")
                nc.scalar.activation(s_sb, g_sb[:, 0:6], AF.Sigmoid)
                t_sb = lsb.tile([128, 2], F32, tag="t")
                nc.scalar.activation(t_sb, g_sb[:, 6:8], AF.Tanh)
                m_sb = lsb.tile([128, 2], F32, tag="m")
                nc.vector.tensor_mul(m_sb, s_sb[:, 0:2], t_sb)
                nc.vector.tensor_mul(c_sb, c_sb, s_sb[:, 2:4])
                nc.vector.tensor_add(c_sb, c_sb, m_sb)
                u_sb = lsb.tile([128, 2], F32, tag="u")
                nc.scalar.activation(u_sb, c_sb, AF.Tanh)
                nc.vector.tensor_mul(hist[:, :, t + 1], u_sb, s_sb[:, 4:6])

            def pre1_block(b):
                for mc in range(8):
                    ps = mmps.tile([128, 512], F32, tag="mm")
                    for kc in range(2):
                        ti = kc * 8 + mc
                        nc.tensor.matmul(
                            ps[:, :BLK],
                            wih1t[:, ti * 128:(ti + 1) * 128],
                            hist0[:, kc, 1 + b * BLK:1 + (b + 1) * BLK],
                            start=(kc == 0), stop=(kc == 1))
                    nc.vector.tensor_scalar_add(
                        pre1[:, mc, b * BLK:(b + 1) * BLK], ps[:, :BLK],
                        b1[:, mc:mc + 1])

            for b in range(NBLK):
                if b == 0:
                    for i in range(BLK):
                        lstm_step(0, i)
                else:
                    for i in range(BLK):
                        lstm_step(0, b * BLK + i)
                        lstm_step(1, (b - 1) * BLK + i)
                pre1_block(b)
            for i in range(BLK):
                lstm_step(1, (NBLK - 1) * BLK + i)

            # dec out (for debugging / istrain=0 fallback)
            nc.sync.dma_start(d_dec, hist1[:, :, 1:N + 1])

            # ---- P^T = Wy1a @ dec.T + Wy1b @ x.T  (dec part fp16) ----
            for lc in range(5):
                for jh in range(2):
                    ps = mmps.tile([128, 512], F32, tag="mm")
                    for kc in range(2):
                        ti = kc * 5 + lc
                        nc.tensor.matmul(
                            ps[:, :320],
                            wy1at[:, ti * 128:(ti + 1) * 128],
                            hist1[:, kc, 1 + jh * 320:1 + (jh + 1) * 320],
                            start=(kc == 0), stop=False)
                    for kc in range(4):
                        ti = kc * 5 + lc
                        nc.tensor.matmul(
                            ps[:, :320],
                            wy1bt[:, ti * 128:(ti + 1) * 128],
                            xt[:, kc, jh * 320:(jh + 1) * 320],
                            start=False, stop=(kc == 3))
                    nc.vector.tensor_copy(
                        pT[:, lc, jh * 320:(jh + 1) * 320], ps[:, :320])

            # ---- per-core shard selection (dynamic DMAs) ----
            base = nc.partition_id() * RPC
            for lc in range(5):
                nc.sync.dma_start(pb[:, lc, :], pT[:, lc, bass.ds(base, RPC)])
            for kc in range(2):
                nc.sync.dma_start(decsel[:, kc, :],
                                  hist1[:, kc, bass.ds(base + 1, RPC)])

            lstm_ctx.close()

            # ---- pointer-score grid: 80 rows/core ----
            grid_ctx = ctx.enter_context(contextlib.ExitStack())
            hidp = grid_ctx.enter_context(tc.tile_pool(name="hidp", bufs=3))
            scps = grid_ctx.enter_context(
                tc.tile_pool(name="scps", bufs=4, space="PSUM"))
            scsb = grid_ctx.enter_context(tc.tile_pool(name="scsb", bufs=3))
            for r in range(RPC):
                ps0 = scps.tile([1, 320], F32, tag="sc")
                ps1 = scps.tile([1, 320], F32, tag="sc")
                for lc in range(5):
                    hid = hidp.tile([128, N], F16, tag="hid")
                    nc.scalar.activation(hid, qT[:, lc, :], AF.Tanh,
                                         bias=pb[:, lc, r:r + 1])
                    nc.tensor.matmul(ps0, wy2c[:, lc:lc + 1],
                                     hid[:, 0:320],
                                     start=(lc == 0), stop=(lc == 4))
                    nc.tensor.matmul(ps1, wy2c[:, lc:lc + 1],
                                     hid[:, 320:640],
                                     start=(lc == 0), stop=(lc == 4))
                srow = scsb.tile([1, N], F32, tag="srow")
                nc.vector.tensor_scalar_add(srow[:, 0:320], ps0, by2_val)
                nc.vector.tensor_scalar_add(srow[:, 320:640], ps1, by2_val)
                nc.sync.dma_start(d_scores[r:r + 1, :], srow)

            grid_ctx.close()

            # ---- label MLP ----
            labps = ctx.enter_context(
                tc.tile_pool(name="labps", bufs=2, space="PSUM"))
            smps = ctx.enter_context(
                tc.tile_pool(name="smps", bufs=1, space="PSUM"))
            for lc in range(5):
                ps = labps.tile([128, RPC], F32, tag="lab")
                for kc in range(8):
                    ti = kc * 5 + lc
                    nc.tensor.matmul(
                        ps, wxct[:, ti * 128:(ti + 1) * 128],
                        xxtsel[:, kc, :],
                        start=(kc == 0), stop=False)
                for kc in range(2):
                    ti = kc * 5 + lc
                    nc.tensor.matmul(
                        ps, wzdect[:, ti * 128:(ti + 1) * 128],
                        decsel[:, kc, :],
                        start=False, stop=(kc == 1))
                nc.scalar.activation(hz[:, lc, :], ps, AF.Relu,
                                     bias=bz1c[:, lc:lc + 1])

            lt_ps = smps.tile([36, RPC], F32, tag="lt")
            for lc in range(5):
                nc.tensor.matmul(lt_ps,
                                 wz2t[:, lc * 36:(lc + 1) * 36],
                                 hz[:, lc, :],
                                 start=(lc == 0), stop=(lc == 4))
            labT = consts.tile([36, RPC], F32, tag="labT")
            nc.scalar.activation(labT, lt_ps, AF.Identity,
                                 bias=bz2c[0:36, 0:1])
            nc.sync.dma_start(d_labelsT, labT)

            tr_ps = smps.tile([RPC, 36], F32, tag="tr")
            nc.tensor.transpose(tr_ps, labT, ident[0:36, 0:36])
            lab_i = consts.tile([RPC, 36], F32, tag="lab_i")
            nc.vector.tensor_copy(lab_i, tr_ps)
            mx = consts.tile([RPC, 8], F32, tag="mx")
            nc.vector.max(mx, lab_i)
            idx = consts.tile([RPC, 8], DT.uint32, tag="idx")
            nc.vector.max_index(idx, mx, lab_i)
            lidx = consts.tile([RPC, 1], DT.int32, tag="lidx")
            nc.vector.tensor_copy(lidx, idx[:, 0:1])
            nc.sync.dma_start(d_lidx, lidx)

    nc.compile()
    _split_drain_waits(nc)
    return nc


def prepare_inputs(inputs):
    """Host-side marshaling: layout transforms only (transpose/cast/concat/
    index-gather of inputs).  Returns (in_maps, by2_val, x, point_idx)."""
    x = _f(inputs["inputs"])[:, 0, :]                     # [640, 512]
    point_idx = np.asarray(inputs["point_idx"])
    istrain = int(np.asarray(inputs["istrain"]))

    whh0 = _perm_rows(_f(inputs["Whh0"]))                 # [1024, 256]
    whh1 = _perm_rows(_f(inputs["Whh1"]))
    wih0 = _perm_rows(_f(inputs["Wih0"]))                 # [1024, 512]
    wih1 = _perm_rows(_f(inputs["Wih1"]))                 # [1024, 256]
    b0 = _perm_rows(_f(inputs["bih0"]) + _f(inputs["bhh0"]))   # [1024]
    b1 = _perm_rows(_f(inputs["bih1"]) + _f(inputs["bhh1"]))

    wy1 = _f(inputs["Wy1"])                               # [640, 1280]
    wy1a = wy1[:, :H]                                     # dec part
    wy1b = wy1[:, H:3 * H]                                # x part
    wy1c = wy1[:, 3 * H:]                                 # prime part
    by1 = _f(inputs["by1"])
    wy2 = _f(inputs["wy2"])
    by2_val = float(np.asarray(inputs["by2"]))

    wz1 = _f(inputs["Wz1"])                               # [640, 1280]
    bz1 = _f(inputs["bz1"])
    wz2 = _f(inputs["Wz2"])                               # [36, 640]
    bz2_val = float(np.asarray(inputs["bz2"])) if np.asarray(
        inputs["bz2"]).ndim == 0 else 0.0

    xg = x[point_idx]                                     # [640, 512]
    xx = np.concatenate([x, xg], axis=1)                  # [640, 1024]
    wxc = np.concatenate([wz1[:, :512], wz1[:, 768:1280]], axis=1)
    wzdec = wz1[:, 512:768]                               # [640, 256]

    rep = {
        "whh0t": _pack_lhsT(whh0.T, 2, 8, np.float16),
        "whh1t": _pack_lhsT(whh1.T, 2, 8, np.float16),
        "wih1t": _pack_lhsT(wih1.T, 2, 8, np.float16),
        "wih0t": _pack_lhsT(wih0.T, 4, 8, np.float32),
        "b0": _f(b0.reshape(8, 128).T),
        "b1": _f(b1.reshape(8, 128).T),
        "xt": _f(x.T.reshape(4, 128, N)),
        "wy1at": _pack_lhsT(wy1a.T, 2, 5, np.float16),
        "wy1bt": _pack_lhsT(wy1b.T, 4, 5, np.float32),
        "wy1ct": _pack_lhsT(wy1c.T, 4, 5, np.float32),
        "by1c": _f(by1.reshape(5, 128).T),
        "wy2c": _f(wy2.reshape(5, 128).T, np.float16),
        "wxct": _pack_lhsT(wxc.T, 8, 5, np.float32),
        "wzdect": _pack_lhsT(wzdec.T, 2, 5, np.float16),
        "bz1c": _f(bz1.reshape(5, 128).T),
        "wz2t": _f(np.concatenate(
            [wz2.T[lc * 128:(lc + 1) * 128, :] for lc in range(5)], axis=1)),
        "bz2c": np.full((36, 1), bz2_val, np.float32),
        "identf": np.eye(128, dtype=np.float32),
    }
    xxt = np.ascontiguousarray(xx.T)                      # [1024, 640]
    in_maps = []
    for k in range(NCORES):
        m = dict(rep)
        m["xxtsel"] = _f(
            xxt[:, k * RPC:(k + 1) * RPC].reshape(8, 128, RPC))
        in_maps.append(m)
    return in_maps, by2_val, x, point_idx, istrain


def assemble(results, point_idx):
    scores = np.concatenate(
        [results[k]["scores_part"] for k in range(NCORES)], axis=0)
    labels = np.concatenate(
        [results[k]["labelsT_part"].T for k in range(NCORES)], axis=0)
    labels_idx = np.concatenate(
        [results[k]["lidx_part"][:, 0] for k in range(NCORES)], axis=0)
    dec = results[0]["dec_out"]      # [128, 2, N] fp16
    dec_full = np.empty((N, H), np.float32)
    dec_full[:, 0:128] = dec[:, 0, :].T.astype(np.float32)
    dec_full[:, 128:256] = dec[:, 1, :].T.astype(np.float32)
    return scores, labels, labels_idx.astype(np.int32), dec_full


def kernel(**inputs):
    in_maps, by2_val, x, point_idx, istrain = prepare_inputs(inputs)
    nc = build_program(by2_val)
    res = bass_utils.run_bass_kernel_spmd(
        nc, in_maps, core_ids=list(range(NCORES)))
    scores, labels, labels_idx, dec = assemble(res.results, point_idx)

    if istrain == 1:
        p_idx = np.asarray(point_idx)
        return (p_idx, scores, labels, labels_idx)

    # istrain != 1 fallback: p_idx = argmax(scores); recompute the (cheap)
    # label MLP on host from the device-computed dec.
    p_idx = np.argmax(scores, axis=1).astype(point_idx.dtype)
    wz1 = _f(inputs["Wz1"]); bz1 = _f(inputs["bz1"])
    wz2 = _f(inputs["Wz2"]); bz2 = _f(inputs["bz2"])
    vec = np.concatenate([x, dec, x[p_idx]], axis=1)
    labels = np.maximum(vec @ wz1.T + bz1, 0.0) @ wz2.T + bz2
    labels_idx = np.argmax(labels, axis=1).astype(np.int32)
    return (p_idx, scores, labels, labels_idx)


# revision 23
# speedup vs baseline: 1.0588x; 1.0588x over previous
"""Trainium2 Bass kernel for nn_Decoder_31044023616500.

Pointer-network decoder: 2-layer LSTM over N=640 steps, an N x N x L tanh
pointer-score grid, and a label MLP.  Runs SPMD on 8 NeuronCores.

Structure:
  - The sequential 2-layer LSTM is replicated on every core; the two layer
    passes are software-pipelined (layer-1 trails layer-0 by 32 steps) so
    their serial chains overlap on different engines.
  - The N x N pointer-score grid is sharded round-robin (core k owns rows
    i = 8j + k), so each core's grid rows become computable progressively
    as the layer-1 (dec) states stream out; grid work is interleaved into
    the LSTM's idle engine slots via a micro-task queue.
  - Per-core row selection is data-driven (one-hot select matmuls + a
    per-core gathered input), keeping the SPMD program fully static.

Precision (validated vs the fp32 reference in numpy):
  fp16 LSTM weights/history + fp32 gates/cell, fp16 grid tanh/reduce,
  fp32 elsewhere => scores abs err ~1.2e-3 (rel ~2.5e-4), argmax exact.
"""

import os
import sys

sys.path.insert(0, "/opt/trn_rl_repo")

import contextlib

import numpy as np

import concourse.bass as bass
import concourse.bacc as bacc
import concourse.tile as tile
from concourse import mybir
from concourse import bass_utils

DT = mybir.dt
F32 = DT.float32
F16 = DT.float16
AF = mybir.ActivationFunctionType

N = 640          # sequence length
H = 256          # lstm width
NCORES = 8
RPC = N // NCORES  # rows per core = 80
BLK = 32           # LSTM pass-interleave block
NBLK = N // BLK    # 20
NCH = 5            # i-chunks of 128 rows
RPCH = 16          # shard rows per chunk (128 / 8)

# gate chunk order in PSUM columns: [i0 i1 f0 f1 o0 o1 g0 g1]
# (sigmoid on cols 0:6, tanh on cols 6:8); original row order is i,f,g,o.
GATE_PERM = [0, 1, 2, 3, 6, 7, 4, 5]


def _perm_rows(w, scale_g=False):
    chunks = w.reshape(8, 128, *w.shape[1:])
    out = np.concatenate([chunks[c] for c in GATE_PERM], axis=0)
    if scale_g:
        out = out.copy()
        out[768:1024] *= 2.0   # tanh(x) = 2*sigmoid(2x) - 1
    return out


def _pack_lhsT(wt, n_k, n_m, dtype):
    """wt: [K, M] (transposed weight) -> [128, n_k*n_m*128] with tile
    (kc, mc) at column block kc*n_m + mc."""
    K, M = wt.shape
    assert K == n_k * 128 and M == n_m * 128
    out = np.empty((128, n_k * n_m * 128), dtype)
    for kc in range(n_k):
        for mc in range(n_m):
            ti = kc * n_m + mc
            out[:, ti * 128:(ti + 1) * 128] = wt[kc * 128:(kc + 1) * 128,
                                                 mc * 128:(mc + 1) * 128]
    return np.ascontiguousarray(out)


def _f(a, dtype=np.float32):
    return np.ascontiguousarray(np.asarray(a, dtype=dtype))


def _split_drain_waits(nc):
    """This walrus build supports only ONE sync-wait on CTRL-class (Drain)
    instructions; Tile's kernel-tail drain carries one wait per active
    proc.  Hoist excess waits onto inserted single-wait drains."""
    fn = nc.m.functions[0]
    for b in fn.blocks:
        insts = list(b.instructions)
        out = []
        changed = False
        for inst in insts:
            si = inst.sync_info
            if inst.opcode == "Drain" and si and si.on_wait and \
                    len(si.on_wait) > 1:
                changed = True
                waits = list(si.on_wait)
                for w in waits[:-1]:
                    d = mybir.InstDrain(
                        name=nc.get_next_instruction_name(),
                        ins=[], outs=[], bass_is_fusable=False)
                    d.engine = inst.engine
                    d.sync_info = mybir.SyncInfo(on_wait=[w], on_update=[])
                    out.append(d)
                si.on_wait = waits[-1:]
            out.append(inst)
        if changed:
            b.instructions = out


def build_program(by2_val: float):
    nc = bacc.Bacc("TRN2", target_bir_lowering=False, debug=False,
                   num_devices=NCORES)

    def din(name, shape, dt):
        return nc.dram_tensor(name, shape, dt, kind="ExternalInput").ap()

    def dout(name, shape, dt):
        return nc.dram_tensor(name, shape, dt, kind="ExternalOutput").ap()

    # ---- DRAM inputs ----
    d_whh0t = din("whh0t", [128, 16 * 128], F16)
    d_whh1t = din("whh1t", [128, 16 * 128], F16)
    d_wih1t = din("wih1t", [128, 16 * 128], F16)
    d_wih0t = din("wih0t", [128, 32 * 128], F16)
    d_b0 = din("b0", [128, 8], F32)
    d_b1 = din("b1", [128, 8], F32)
    d_xt = din("xt", [4, 128, N], F32)
    d_xt16 = din("xt16", [4, 128, N], F16)
    d_wy1ar = din("wy1ar", [2, 128, N], F16)
    d_wy1br = din("wy1br", [4, 128, N], F32)
    d_wy1ct = din("wy1ct", [128, 20 * 128], F32)
    d_by1c = din("by1c", [128, 5], F32)
    d_wy2c = din("wy2c", [128, 5], F16)
    d_wxct = din("wxct", [128, 40 * 128], F32)
    d_wzdect = din("wzdect", [128, 10 * 128], F16)
    d_bz1c = din("bz1c", [128, 5], F32)
    d_wz2t = din("wz2t", [128, 5 * 36], F32)
    d_bz2c = din("bz2c", [36, 1], F32)
    d_ident = din("identf", [128, 128], F32)
    d_ident16 = din("ident16", [128, 128], F16)
    d_onehot = din("onehot", [128, RPCH], F32)      # per-core
    d_xxtsel = din("xxtsel", [8, 128, RPC], F32)    # per-core

    # ---- DRAM outputs (per core; row j <-> global row 8j+core) ----
    d_scores = dout("scores_part", [RPC, N], F32)
    d_labelsT = dout("labelsT_part", [36, RPC], F32)
    d_lidx = dout("lidx_part", [RPC, 1], DT.int32)
    d_dec = dout("dec_out", [128, 2, N], F16)

    with tile.TileContext(nc) as tc:
        ctx = contextlib.ExitStack()
        with ctx:
            consts = ctx.enter_context(tc.tile_pool(name="consts", bufs=1))

            def cload(dram, shape, dt, tag):
                t = consts.tile(shape, dt, tag=tag)
                nc.sync.dma_start(t, dram)
                return t

            whh0t = cload(d_whh0t, [128, 2048], F16, "whh0t")
            whh1t = cload(d_whh1t, [128, 2048], F16, "whh1t")
            wih1t = cload(d_wih1t, [128, 2048], F16, "wih1t")
            wih0t = cload(d_wih0t, [128, 4096], F16, "wih0t")
            b0 = cload(d_b0, [128, 8], F32, "b0")
            b1 = cload(d_b1, [128, 8], F32, "b1")
            xt = consts.tile([128, 4, N], F32, tag="xt")
            for kc in range(4):
                nc.sync.dma_start(xt[:, kc, :], d_xt[kc, :, :])
            xt16 = consts.tile([128, 4, N], F16, tag="xt16")
            for kc in range(4):
                nc.sync.dma_start(xt16[:, kc, :], d_xt16[kc, :, :])
            wy1ar = consts.tile([128, 2, N], F16, tag="wy1ar")
            for kc in range(2):
                nc.sync.dma_start(wy1ar[:, kc, :], d_wy1ar[kc, :, :])
            wy1br = consts.tile([128, 4, N], F32, tag="wy1br")
            for kc in range(4):
                nc.sync.dma_start(wy1br[:, kc, :], d_wy1br[kc, :, :])
            wy1ct = cload(d_wy1ct, [128, 2560], F32, "wy1ct")
            by1c = cload(d_by1c, [128, 5], F32, "by1c")
            wy2c = cload(d_wy2c, [128, 5], F16, "wy2c")
            wxct = cload(d_wxct, [128, 5120], F32, "wxct")
            wzdect = cload(d_wzdect, [128, 1280], F16, "wzdect")
            bz1c = cload(d_bz1c, [128, 5], F32, "bz1c")
            wz2t = cload(d_wz2t, [128, 180], F32, "wz2t")
            bz2c = cload(d_bz2c, [36, 1], F32, "bz2c")
            ident = cload(d_ident, [128, 128], F32, "identf")
            ident16 = cload(d_ident16, [128, 128], F16, "ident16")
            onehot = cload(d_onehot, [128, RPCH], F32, "onehot")
            xxtsel = consts.tile([128, 8, RPC], F32, tag="xxtsel")
            for kc in range(8):
                nc.sync.dma_start(xxtsel[:, kc, :], d_xxtsel[kc, :, :])

            hist0 = consts.tile([128, 2, N + 1], F16, tag="hist0")
            hist1 = consts.tile([128, 2, N + 1], F16, tag="hist1")
            c0 = consts.tile([128, 2], F32, tag="c0")
            c1 = consts.tile([128, 2], F32, tag="c1")
            pre0 = consts.tile([128, 8, N], F16, tag="pre0")
            pre1 = consts.tile([128, 8, N], F16, tag="pre1")
            pi = consts.tile([128, NCH, N], F32, tag="pi")   # P, i-major
            qT = consts.tile([128, 5, N], F32, tag="qT")
            pb = consts.tile([128, 5, RPC], F32, tag="pb")
            decsel = consts.tile([128, 2, RPC], F16, tag="decsel")
            hz = consts.tile([128, 5, RPC], F32, tag="hz")

            nc.vector.memset(hist0[:, :, 0], 0.0)
            nc.vector.memset(hist1[:, :, 0], 0.0)
            nc.vector.memset(c0, 0.0)
            nc.vector.memset(c1, 0.0)

            main_ctx = ctx.enter_context(contextlib.ExitStack())
            mmps = main_ctx.enter_context(
                tc.tile_pool(name="mmps", bufs=2, space="PSUM"))
            gps = main_ctx.enter_context(
                tc.tile_pool(name="gps", bufs=3, space="PSUM"))
            lsb = main_ctx.enter_context(tc.tile_pool(name="lsb", bufs=3))
            hidp = main_ctx.enter_context(tc.tile_pool(name="hidp", bufs=3))
            scps = main_ctx.enter_context(
                tc.tile_pool(name="scps", bufs=2, space="PSUM"))
            scsb = main_ctx.enter_context(tc.tile_pool(name="scsb", bufs=3))

            # ---- pre0 = Wih0 @ x.T + b0  (fp16 weights, fp32 psum) ----
            for mc in range(8):
                for n0, nsz in ((0, 512), (512, 128)):
                    ps = mmps.tile([128, 512], F32, tag="mm")
                    for kc in range(4):
                        ti = kc * 8 + mc
                        nc.tensor.matmul(
                            ps[:, :nsz],
                            wih0t[:, ti * 128:(ti + 1) * 128],
                            xt16[:, kc, n0:n0 + nsz],
                            start=(kc == 0), stop=(kc == 3))
                    nc.vector.tensor_scalar_add(
                        pre0[:, mc, n0:n0 + nsz], ps[:, :nsz],
                        b0[:, mc:mc + 1])

            # Q and P(x-part) precomputes run as early queue tasks
            def q_task(lc, jh):
                def run():
                    ps = mmps.tile([128, 512], F32, tag="mm",
                                   name=f"qps_{lc}_{jh}")
                    for kc in range(4):
                        ti = kc * 5 + lc
                        nc.tensor.matmul(
                            ps[:, :320],
                            wy1ct[:, ti * 128:(ti + 1) * 128],
                            xt[:, kc, jh * 320:(jh + 1) * 320],
                            start=(kc == 0), stop=(kc == 3))
                    nc.vector.tensor_scalar_add(
                        qT[:, lc, jh * 320:(jh + 1) * 320], ps[:, :320],
                        by1c[:, lc:lc + 1])
                return run

            def px_task(c, jh):
                def run():
                    ps = mmps.tile([128, 512], F32, tag="mm",
                                   name=f"pxps_{c}_{jh}")
                    for kc in range(4):
                        nc.tensor.matmul(
                            ps[:, :320],
                            xt[:, kc, c * 128:(c + 1) * 128],
                            wy1br[:, kc, jh * 320:(jh + 1) * 320],
                            start=(kc == 0), stop=(kc == 3))
                    nc.vector.tensor_copy(
                        pi[:, c, jh * 320:(jh + 1) * 320], ps[:, :320])
                return run

            # ---- LSTM machinery ----
            def lstm_step(layer, t):
                hist, c_sb, pre, wt = ((hist0, c0, pre0, whh0t) if layer == 0
                                       else (hist1, c1, pre1, whh1t))
                g_ps = gps.tile([128, 8], F32, tag="g")
                nc.tensor.matmul(g_ps, ident16, pre[:, :, t],
                                 start=True, stop=False,
                                 skip_group_check=True)
                for mc in range(8):
                    for kc in range(2):
                        ti = kc * 8 + mc
                        nc.tensor.matmul(
                            g_ps[:, mc:mc + 1],
                            wt[:, ti * 128:(ti + 1) * 128],
                            hist[:, kc, t:t + 1],
                            start=False, stop=(kc == 1),
                            skip_group_check=True)
                s_sb = lsb.tile([128, 8], F32, tag="s")
                nc.scalar.activation(s_sb, g_ps, AF.Sigmoid)
                w_sb = lsb.tile([128, 2], F32, tag="w")
                nc.vector.tensor_scalar(w_sb, s_sb[:, 6:8], 2.0, -1.0,
                                        mybir.AluOpType.mult,
                                        mybir.AluOpType.add)
                m_sb = lsb.tile([128, 2], F32, tag="m")
                nc.vector.tensor_mul(m_sb, s_sb[:, 0:2], w_sb)
                nc.vector.tensor_mul(c_sb, c_sb, s_sb[:, 2:4])
                nc.vector.tensor_add(c_sb, c_sb, m_sb)
                u_sb = lsb.tile([128, 2], F32, tag="u")
                nc.scalar.activation(u_sb, c_sb, AF.Tanh)
                nc.vector.tensor_mul(hist[:, :, t + 1], u_sb, s_sb[:, 4:6])

            def pre1_block(b):
                for mc in range(8):
                    ps = mmps.tile([128, 512], F32, tag="mm")
                    for kc in range(2):
                        ti = kc * 8 + mc
                        nc.tensor.matmul(
                            ps[:, :BLK],
                            wih1t[:, ti * 128:(ti + 1) * 128],
                            hist0[:, kc, 1 + b * BLK:1 + (b + 1) * BLK],
                            start=(kc == 0), stop=(kc == 1))
                    nc.vector.tensor_scalar_add(
                        pre1[:, mc, b * BLK:(b + 1) * BLK], ps[:, :BLK],
                        b1[:, mc:mc + 1])

            # ---- grid micro-tasks (one ~0.7us ACT slot per LSTM pair) ----
            tasks = []
            row_state = {}

            def grid_task(j, lc):
                def run():
                    if lc == 0:
                        row_state[j] = (
                            scps.tile([1, 320], F32, tag="sc",
                                      name=f"sc0_{j}"),
                            scps.tile([1, 320], F32, tag="sc",
                                      name=f"sc1_{j}"))
                    ps0, ps1 = row_state[j]
                    hid = hidp.tile([128, N], F16, tag="hid")
                    nc.scalar.activation(hid, qT[:, lc, :], AF.Tanh,
                                         bias=pb[:, lc, j:j + 1])
                    nc.tensor.matmul(ps0, wy2c[:, lc:lc + 1],
                                     hid[:, 0:320],
                                     start=(lc == 0), stop=(lc == 4))
                    nc.tensor.matmul(ps1, wy2c[:, lc:lc + 1],
                                     hid[:, 320:640],
                                     start=(lc == 0), stop=(lc == 4))
                return run

            def grid_epilogue(j):
                def run():
                    ps0, ps1 = row_state.pop(j)
                    srow = scsb.tile([1, N], F32, tag="srow")
                    nc.vector.tensor_scalar_add(srow[:, 0:320], ps0, by2_val)
                    nc.vector.tensor_scalar_add(srow[:, 320:640], ps1,
                                                by2_val)
                    nc.sync.dma_start(d_scores[j:j + 1, :], srow)
                return run

            def emit_chunk(c, i0=0, i1=128, jj0=0, jj1=RPCH):
                # P dec part for rows [c*128+i0, c*128+i1)
                for jh in range(2):
                    ps = mmps.tile([128, 512], F32, tag="mm",
                                   name=f"pdec_{c}_{i0}_{jh}")
                    for kc in range(2):
                        nc.tensor.matmul(
                            ps[i0:i1, :320],
                            hist1[:, kc,
                                  1 + c * 128 + i0:1 + c * 128 + i1],
                            wy1ar[:, kc, jh * 320:(jh + 1) * 320],
                            start=(kc == 0), stop=(kc == 1))
                    nc.vector.tensor_add(
                        pi[i0:i1, c, jh * 320:(jh + 1) * 320],
                        pi[i0:i1, c, jh * 320:(jh + 1) * 320],
                        ps[i0:i1, :320])
                # select this core's rows jj0..jj1 of chunk c into pb
                # (one-hot columns only touch ready dec rows)
                nsel = jj1 - jj0
                for lc in range(5):
                    ps = mmps.tile([128, 512], F32, tag="mm",
                                   name=f"psel_{c}_{i0}_{lc}")
                    nc.tensor.matmul(ps[:, :nsel],
                                     pi[:, c, lc * 128:(lc + 1) * 128],
                                     onehot[:, jj0:jj1],
                                     start=True, stop=True)
                    nc.vector.tensor_copy(
                        pb[:, lc, c * RPCH + jj0:c * RPCH + jj1],
                        ps[:, :nsel])
                if os.environ.get("K_NOGRID") != "1":
                    for jj in range(jj0, jj1):
                        j = c * RPCH + jj
                        for lc in range(5):
                            tasks.append(grid_task(j, lc))
                        tasks.append(grid_epilogue(j))

            def drain_task():
                if tasks:
                    tasks.pop(0)()
                if len(tasks) > 24:
                    tasks.pop(0)()

            # ---- main pipeline ----
            for lc in range(5):
                for jh in range(2):
                    tasks.append(q_task(lc, jh))
            for c in range(NCH):
                for jh in range(2):
                    tasks.append(px_task(c, jh))
            for b in range(NBLK):
                for i in range(BLK):
                    lstm_step(0, b * BLK + i)
                    if b >= 1:
                        lstm_step(1, (b - 1) * BLK + i)
                    drain_task()
                pre1_block(b)
                if b >= 4 and b % 4 == 0:
                    emit_chunk(b // 4 - 1)
                elif b == 18:
                    emit_chunk(NCH - 1, 0, 64, 0, RPCH // 2)
            for i in range(BLK):
                lstm_step(1, (NBLK - 1) * BLK + i)
                drain_task()
            emit_chunk(NCH - 1, 64, 128, RPCH // 2, RPCH)
            while tasks:
                drain_task()

            nc.sync.dma_start(d_dec, hist1[:, :, 1:N + 1])

            # this core's dec columns (i = 8j + core), via strided dyn DMA
            pid = nc.partition_id()
            hist1_r = hist1[:, :, 1:N + 1].rearrange(
                "p h (j e) -> p h j e", e=8)
            for kc in range(2):
                nc.sync.dma_start(
                    decsel[:, kc, :],
                    hist1_r[:, kc, :, bass.ds(pid, 1)])

            main_ctx.close()

            # ---- label MLP (tail) ----
            labps = ctx.enter_context(
                tc.tile_pool(name="labps", bufs=2, space="PSUM"))
            smps = ctx.enter_context(
                tc.tile_pool(name="smps", bufs=1, space="PSUM"))
            for lc in range(5):
                ps = labps.tile([128, RPC], F32, tag="lab")
                for kc in range(8):
                    ti = kc * 5 + lc
                    nc.tensor.matmul(
                        ps, wxct[:, ti * 128:(ti + 1) * 128],
                        xxtsel[:, kc, :],
                        start=(kc == 0), stop=False)
                for kc in range(2):
                    ti = kc * 5 + lc
                    nc.tensor.matmul(
                        ps, wzdect[:, ti * 128:(ti + 1) * 128],
                        decsel[:, kc, :],
                        start=False, stop=(kc == 1))
                nc.scalar.activation(hz[:, lc, :], ps, AF.Relu,
                                     bias=bz1c[:, lc:lc + 1])

            lt_ps = smps.tile([36, RPC], F32, tag="lt")
            for lc in range(5):
                nc.tensor.matmul(lt_ps,
                                 wz2t[:, lc * 36:(lc + 1) * 36],
                                 hz[:, lc, :],
                                 start=(lc == 0), stop=(lc == 4))
            labT = consts.tile([36, RPC], F32, tag="labT")
            nc.scalar.activation(labT, lt_ps, AF.Identity,
                                 bias=bz2c[0:36, 0:1])
            nc.sync.dma_start(d_labelsT, labT)

            tr_ps = smps.tile([RPC, 36], F32, tag="tr")
            nc.tensor.transpose(tr_ps, labT, ident[0:36, 0:36])
            lab_i = consts.tile([RPC, 36], F32, tag="lab_i")
            nc.vector.tensor_copy(lab_i, tr_ps)
            mx = consts.tile([RPC, 8], F32, tag="mx")
            nc.vector.max(mx, lab_i)
            idx = consts.tile([RPC, 8], DT.uint32, tag="idx")
            nc.vector.max_index(idx, mx, lab_i)
            lidx = consts.tile([RPC, 1], DT.int32, tag="lidx")
            nc.vector.tensor_copy(lidx, idx[:, 0:1])
            nc.sync.dma_start(d_lidx, lidx)

    nc.compile()
    _split_drain_waits(nc)
    return nc


def prepare_inputs(inputs):
    x = _f(inputs["inputs"])[:, 0, :]                     # [640, 512]
    point_idx = np.asarray(inputs["point_idx"])
    istrain = int(np.asarray(inputs["istrain"]))

    whh0 = _perm_rows(_f(inputs["Whh0"]), True)
    whh1 = _perm_rows(_f(inputs["Whh1"]), True)
    wih0 = _perm_rows(_f(inputs["Wih0"]), True)
    wih1 = _perm_rows(_f(inputs["Wih1"]), True)
    b0 = _perm_rows(_f(inputs["bih0"]) + _f(inputs["bhh0"]), True)
    b1 = _perm_rows(_f(inputs["bih1"]) + _f(inputs["bhh1"]), True)

    wy1 = _f(inputs["Wy1"])
    wy1a, wy1b, wy1c = wy1[:, :H], wy1[:, H:3 * H], wy1[:, 3 * H:]
    by1 = _f(inputs["by1"])
    wy2 = _f(inputs["wy2"])
    by2_val = float(np.asarray(inputs["by2"]))

    wz1 = _f(inputs["Wz1"])
    bz1 = _f(inputs["bz1"])
    wz2 = _f(inputs["Wz2"])
    bz2 = _f(inputs["bz2"]).reshape(-1)
    if bz2.size == 1:
        bz2 = np.full(36, float(bz2[0]), np.float32)

    xg = x[point_idx]
    xx = np.concatenate([x, xg], axis=1)                  # [640, 1024]
    wxc = np.concatenate([wz1[:, :512], wz1[:, 768:1280]], axis=1)
    wzdec = wz1[:, 512:768]

    xtr = np.ascontiguousarray(x.T)                       # [512, 640]
    rep = {
        "whh0t": _pack_lhsT(whh0.T, 2, 8, np.float16),
        "whh1t": _pack_lhsT(whh1.T, 2, 8, np.float16),
        "wih1t": _pack_lhsT(wih1.T, 2, 8, np.float16),
        "wih0t": _pack_lhsT(wih0.T, 4, 8, np.float16),
        "b0": _f(b0.reshape(8, 128).T),
        "b1": _f(b1.reshape(8, 128).T),
        "xt": _f(xtr.reshape(4, 128, N)),
        "xt16": _f(xtr.reshape(4, 128, N), np.float16),
        "wy1ar": _f(wy1a.T.reshape(2, 128, N), np.float16),
        "wy1br": _f(wy1b.T.reshape(4, 128, N)),
        "wy1ct": _pack_lhsT(wy1c.T, 4, 5, np.float32),
        "by1c": _f(by1.reshape(5, 128).T),
        "wy2c": _f(wy2.reshape(5, 128).T, np.float16),
        "wxct": _pack_lhsT(wxc.T, 8, 5, np.float32),
        "wzdect": _pack_lhsT(wzdec.T, 2, 5, np.float16),
        "bz1c": _f(bz1.reshape(5, 128).T),
        "wz2t": _f(np.concatenate(
            [wz2.T[lc * 128:(lc + 1) * 128, :] for lc in range(5)], axis=1)),
        "bz2c": _f(bz2.reshape(36, 1)),
        "identf": np.eye(128, dtype=np.float32),
        "ident16": np.eye(128, dtype=np.float16),
    }
    xxt = np.ascontiguousarray(xx.T)                      # [1024, 640]
    in_maps = []
    for k in range(NCORES):
        m = dict(rep)
        oh = np.zeros((128, RPCH), np.float32)
        for jj in range(RPCH):
            oh[8 * jj + k, jj] = 1.0
        m["onehot"] = oh
        m["xxtsel"] = _f(
            np.ascontiguousarray(xxt[:, k::8]).reshape(8, 128, RPC))
        in_maps.append(m)
    return in_maps, by2_val, x, point_idx, istrain


def assemble(results, point_idx):
    scores = np.empty((N, N), np.float32)
    labels = np.empty((N, 36), np.float32)
    labels_idx = np.empty((N,), np.int32)
    for k in range(NCORES):
        scores[k::8] = results[k]["scores_part"]
        labels[k::8] = results[k]["labelsT_part"].T
        labels_idx[k::8] = results[k]["lidx_part"][:, 0]
    dec = results[0]["dec_out"]
    dec_full = np.empty((N, H), np.float32)
    dec_full[:, 0:128] = dec[:, 0, :].T.astype(np.float32)
    dec_full[:, 128:256] = dec[:, 1, :].T.astype(np.float32)
    return scores, labels, labels_idx, dec_full


def kernel(**inputs):
    in_maps, by2_val, x, point_idx, istrain = prepare_inputs(inputs)
    nc = build_program(by2_val)
    res = bass_utils.run_bass_kernel_spmd(
        nc, in_maps, core_ids=list(range(NCORES)))
    scores, labels, labels_idx, dec = assemble(res.results, point_idx)

    if istrain == 1:
        return (np.asarray(point_idx), scores, labels, labels_idx)

    # istrain != 1 fallback: p_idx = argmax(scores); recompute the (small)
    # label MLP on host from the device-computed dec.
    p_idx = np.argmax(scores, axis=1).astype(np.asarray(point_idx).dtype)
    wz1 = _f(inputs["Wz1"]); bz1 = _f(inputs["bz1"])
    wz2 = _f(inputs["Wz2"]); bz2f = _f(inputs["bz2"])
    vec = np.concatenate([x, dec, x[p_idx]], axis=1)
    labels = np.maximum(vec @ wz1.T + bz1, 0.0) @ wz2.T + bz2f
    labels_idx = np.argmax(labels, axis=1).astype(np.int32)
    return (p_idx, scores, labels, labels_idx)


# revision 24
# speedup vs baseline: 4186.9900x; 3954.2924x over previous
"""Trainium2 Bass kernel for nn_Decoder_31044023616500.

Pointer-network decoder: 2-layer LSTM over N=640 steps, an N x N x L tanh
pointer-score grid, and a label MLP.  Runs SPMD on 8 NeuronCores.

Structure:
  - The sequential 2-layer LSTM is replicated on every core; the two layer
    passes are software-pipelined (layer-1 trails layer-0 by 32 steps) so
    their serial chains overlap on different engines.
  - The N x N pointer-score grid is sharded round-robin (core k owns rows
    i = 8j + k), so each core's grid rows become computable progressively
    as the layer-1 (dec) states stream out; grid work is interleaved into
    the LSTM's idle engine slots via a micro-task queue.
  - Per-core row selection is data-driven (one-hot select matmuls + a
    per-core gathered input), keeping the SPMD program fully static.

Precision (validated vs the fp32 reference in numpy):
  fp16 LSTM weights/history + fp32 gates/cell, fp16 grid tanh/reduce,
  fp32 elsewhere => scores abs err ~1.2e-3 (rel ~2.5e-4), argmax exact.
"""

import os
import sys

sys.path.insert(0, "/opt/trn_rl_repo")

import contextlib

import numpy as np

import concourse.bass as bass
import concourse.bacc as bacc
import concourse.tile as tile
from concourse import mybir
from concourse import bass_utils

DT = mybir.dt
F32 = DT.float32
F16 = DT.float16
AF = mybir.ActivationFunctionType

N = 640          # sequence length
H = 256          # lstm width
NCORES = 8
RPC = N // NCORES  # rows per core = 80
BLK = 32           # LSTM pass-interleave block
NBLK = N // BLK    # 20
NCH = 5            # i-chunks of 128 rows
RPCH = 16          # shard rows per chunk (128 / 8)

# gate chunk order in PSUM columns: [i0 i1 f0 f1 o0 o1 g0 g1]
# (sigmoid on cols 0:6, tanh on cols 6:8); original row order is i,f,g,o.
GATE_PERM = [0, 1, 2, 3, 6, 7, 4, 5]


def _perm_rows(w, scale_g=False):
    chunks = w.reshape(8, 128, *w.shape[1:])
    out = np.concatenate([chunks[c] for c in GATE_PERM], axis=0)
    if scale_g:
        out = out.copy()
        out[768:1024] *= 2.0   # tanh(x) = 2*sigmoid(2x) - 1
    return out


def _pack_lhsT(wt, n_k, n_m, dtype):
    """wt: [K, M] (transposed weight) -> [128, n_k*n_m*128] with tile
    (kc, mc) at column block kc*n_m + mc."""
    K, M = wt.shape
    assert K == n_k * 128 and M == n_m * 128
    out = np.empty((128, n_k * n_m * 128), dtype)
    for kc in range(n_k):
        for mc in range(n_m):
            ti = kc * n_m + mc
            out[:, ti * 128:(ti + 1) * 128] = wt[kc * 128:(kc + 1) * 128,
                                                 mc * 128:(mc + 1) * 128]
    return np.ascontiguousarray(out)


def _f(a, dtype=np.float32):
    return np.ascontiguousarray(np.asarray(a, dtype=dtype))


def _split_drain_waits(nc):
    """This walrus build supports only ONE sync-wait on CTRL-class (Drain)
    instructions; Tile's kernel-tail drain carries one wait per active
    proc.  Hoist excess waits onto inserted single-wait drains."""
    fn = nc.m.functions[0]
    for b in fn.blocks:
        insts = list(b.instructions)
        out = []
        changed = False
        for inst in insts:
            si = inst.sync_info
            if inst.opcode == "Drain" and si and si.on_wait and \
                    len(si.on_wait) > 1:
                changed = True
                waits = list(si.on_wait)
                for w in waits[:-1]:
                    d = mybir.InstDrain(
                        name=nc.get_next_instruction_name(),
                        ins=[], outs=[], bass_is_fusable=False)
                    d.engine = inst.engine
                    d.sync_info = mybir.SyncInfo(on_wait=[w], on_update=[])
                    out.append(d)
                si.on_wait = waits[-1:]
            out.append(inst)
        if changed:
            b.instructions = out


def build_program(by2_val: float):
    nc = bacc.Bacc("TRN2", target_bir_lowering=False, debug=False,
                   num_devices=NCORES)

    def din(name, shape, dt):
        return nc.dram_tensor(name, shape, dt, kind="ExternalInput").ap()

    def dout(name, shape, dt):
        return nc.dram_tensor(name, shape, dt, kind="ExternalOutput").ap()

    # ---- DRAM inputs ----
    d_whh0t = din("whh0t", [128, 16 * 128], F16)
    d_whh1t = din("whh1t", [128, 16 * 128], F16)
    d_wih1t = din("wih1t", [128, 16 * 128], F16)
    d_wih0t = din("wih0t", [128, 32 * 128], F16)
    d_b0 = din("b0", [128, 8], F32)
    d_b1 = din("b1", [128, 8], F32)
    d_xt = din("xt", [4, 128, N], F32)
    d_xt16 = din("xt16", [4, 128, N], F16)
    d_wy1ar = din("wy1ar", [2, 128, N], F16)
    d_wy1br = din("wy1br", [4, 128, N], F32)
    d_wy1ct = din("wy1ct", [128, 20 * 128], F32)
    d_by1c = din("by1c", [128, 5], F32)
    d_wy2c = din("wy2c", [128, 5], F16)
    d_wxct = din("wxct", [128, 40 * 128], F32)
    d_wzdect = din("wzdect", [128, 10 * 128], F16)
    d_bz1c = din("bz1c", [128, 5], F32)
    d_wz2t = din("wz2t", [128, 5 * 36], F32)
    d_bz2c = din("bz2c", [36, 1], F32)
    d_ident = din("identf", [128, 128], F32)
    d_ident16 = din("ident16", [128, 128], F16)
    d_onehot = din("onehot", [128, RPCH], F32)      # per-core
    d_xxtsel = din("xxtsel", [8, 128, RPC], F32)    # per-core

    # ---- DRAM outputs (per core; row j <-> global row 8j+core) ----
    d_scores = dout("scores_part", [RPC, N], F32)
    d_labelsT = dout("labelsT_part", [36, RPC], F32)
    d_lidx = dout("lidx_part", [RPC, 1], DT.int32)
    d_dec = dout("dec_out", [128, 2, N], F16)

    with tile.TileContext(nc) as tc:
        ctx = contextlib.ExitStack()
        with ctx:
            consts = ctx.enter_context(tc.tile_pool(name="consts", bufs=1))

            def cload(dram, shape, dt, tag):
                t = consts.tile(shape, dt, tag=tag)
                nc.sync.dma_start(t, dram)
                return t

            whh0t = cload(d_whh0t, [128, 2048], F16, "whh0t")
            whh1t = cload(d_whh1t, [128, 2048], F16, "whh1t")
            wih1t = cload(d_wih1t, [128, 2048], F16, "wih1t")
            wih0t = cload(d_wih0t, [128, 4096], F16, "wih0t")
            b0 = cload(d_b0, [128, 8], F32, "b0")
            b1 = cload(d_b1, [128, 8], F32, "b1")
            xt16 = consts.tile([128, 4, N], F16, tag="xt16")
            for kc in range(4):
                nc.sync.dma_start(xt16[:, kc, :], d_xt16[kc, :, :])
            ident16 = cload(d_ident16, [128, 128], F16, "ident16")
            xt = consts.tile([128, 4, N], F32, tag="xt")
            for kc in range(4):
                nc.sync.dma_start(xt[:, kc, :], d_xt[kc, :, :])
            wy1ar = consts.tile([128, 2, N], F16, tag="wy1ar")
            for kc in range(2):
                nc.sync.dma_start(wy1ar[:, kc, :], d_wy1ar[kc, :, :])
            by1c = cload(d_by1c, [128, 5], F32, "by1c")
            wy2c = cload(d_wy2c, [128, 5], F16, "wy2c")
            onehot = cload(d_onehot, [128, RPCH], F32, "onehot")
            wy1ct = cload(d_wy1ct, [128, 2560], F32, "wy1ct")
            wy1br = consts.tile([128, 4, N], F32, tag="wy1br")
            for kc in range(4):
                nc.sync.dma_start(wy1br[:, kc, :], d_wy1br[kc, :, :])
            wxct = cload(d_wxct, [128, 5120], F32, "wxct")
            wzdect = cload(d_wzdect, [128, 1280], F16, "wzdect")
            bz1c = cload(d_bz1c, [128, 5], F32, "bz1c")
            wz2t = cload(d_wz2t, [128, 180], F32, "wz2t")
            bz2c = cload(d_bz2c, [36, 1], F32, "bz2c")
            ident = cload(d_ident, [128, 128], F32, "identf")
            xxtsel = consts.tile([128, 8, RPC], F32, tag="xxtsel")
            for kc in range(8):
                nc.sync.dma_start(xxtsel[:, kc, :], d_xxtsel[kc, :, :])

            hist0 = consts.tile([128, 2, N + 1], F16, tag="hist0")
            hist1 = consts.tile([128, 2, N + 1], F16, tag="hist1")
            c0 = consts.tile([128, 2], F32, tag="c0")
            c1 = consts.tile([128, 2], F32, tag="c1")
            pre0 = consts.tile([128, 8, N], F16, tag="pre0")
            pre1 = consts.tile([128, 8, N], F16, tag="pre1")
            pi = consts.tile([128, NCH, N], F32, tag="pi")   # P, i-major
            qT = consts.tile([128, 5, N], F32, tag="qT")
            pb = consts.tile([128, 5, RPC], F32, tag="pb")
            decsel = consts.tile([128, 2, RPC], F16, tag="decsel")
            hz = consts.tile([128, 5, RPC], F32, tag="hz")

            nc.vector.memset(hist0[:, :, 0], 0.0)
            nc.vector.memset(hist1[:, :, 0], 0.0)
            nc.vector.memset(c0, 0.0)
            nc.vector.memset(c1, 0.0)

            main_ctx = ctx.enter_context(contextlib.ExitStack())
            mmps = main_ctx.enter_context(
                tc.tile_pool(name="mmps", bufs=2, space="PSUM"))
            gps = main_ctx.enter_context(
                tc.tile_pool(name="gps", bufs=3, space="PSUM"))
            lsb = main_ctx.enter_context(tc.tile_pool(name="lsb", bufs=3))
            hidp = main_ctx.enter_context(tc.tile_pool(name="hidp", bufs=3))
            scps = main_ctx.enter_context(
                tc.tile_pool(name="scps", bufs=2, space="PSUM"))
            scsb = main_ctx.enter_context(tc.tile_pool(name="scsb", bufs=3))

            # ---- pre0 = Wih0 @ x.T + b0  (fp16 weights, fp32 psum) ----
            for mc in range(8):
                for n0, nsz in ((0, 512), (512, 128)):
                    ps = mmps.tile([128, 512], F32, tag="mm")
                    for kc in range(4):
                        ti = kc * 8 + mc
                        nc.tensor.matmul(
                            ps[:, :nsz],
                            wih0t[:, ti * 128:(ti + 1) * 128],
                            xt16[:, kc, n0:n0 + nsz],
                            start=(kc == 0), stop=(kc == 3))
                    nc.vector.tensor_scalar_add(
                        pre0[:, mc, n0:n0 + nsz], ps[:, :nsz],
                        b0[:, mc:mc + 1])

            # Q and P(x-part) precomputes run as early queue tasks
            def q_task(lc, jh):
                def run():
                    ps = mmps.tile([128, 512], F32, tag="mm",
                                   name=f"qps_{lc}_{jh}")
                    for kc in range(4):
                        ti = kc * 5 + lc
                        nc.tensor.matmul(
                            ps[:, :320],
                            wy1ct[:, ti * 128:(ti + 1) * 128],
                            xt[:, kc, jh * 320:(jh + 1) * 320],
                            start=(kc == 0), stop=(kc == 3))
                    nc.vector.tensor_scalar_add(
                        qT[:, lc, jh * 320:(jh + 1) * 320], ps[:, :320],
                        by1c[:, lc:lc + 1])
                return run

            def px_task(c, jh):
                def run():
                    ps = mmps.tile([128, 512], F32, tag="mm",
                                   name=f"pxps_{c}_{jh}")
                    for kc in range(4):
                        nc.tensor.matmul(
                            ps[:, :320],
                            xt[:, kc, c * 128:(c + 1) * 128],
                            wy1br[:, kc, jh * 320:(jh + 1) * 320],
                            start=(kc == 0), stop=(kc == 3))
                    nc.vector.tensor_copy(
                        pi[:, c, jh * 320:(jh + 1) * 320], ps[:, :320])
                return run

            # ---- LSTM machinery ----
            def lstm_step(layer, t):
                hist, c_sb, pre, wt = ((hist0, c0, pre0, whh0t) if layer == 0
                                       else (hist1, c1, pre1, whh1t))
                g_ps = gps.tile([128, 8], F32, tag="g")
                nc.tensor.matmul(g_ps, ident16, pre[:, :, t],
                                 start=True, stop=False,
                                 skip_group_check=True)
                for mc in range(8):
                    for kc in range(2):
                        ti = kc * 8 + mc
                        nc.tensor.matmul(
                            g_ps[:, mc:mc + 1],
                            wt[:, ti * 128:(ti + 1) * 128],
                            hist[:, kc, t:t + 1],
                            start=False, stop=(kc == 1),
                            skip_group_check=True)
                s_sb = lsb.tile([128, 8], F32, tag="s")
                nc.scalar.activation(s_sb, g_ps, AF.Sigmoid)
                w_sb = lsb.tile([128, 2], F32, tag="w")
                nc.vector.tensor_scalar(w_sb, s_sb[:, 6:8], 2.0, -1.0,
                                        mybir.AluOpType.mult,
                                        mybir.AluOpType.add)
                m_sb = lsb.tile([128, 2], F32, tag="m")
                nc.vector.tensor_mul(m_sb, s_sb[:, 0:2], w_sb)
                nc.vector.tensor_mul(c_sb, c_sb, s_sb[:, 2:4])
                nc.vector.tensor_add(c_sb, c_sb, m_sb)
                u_sb = lsb.tile([128, 2], F32, tag="u")
                nc.scalar.activation(u_sb, c_sb, AF.Tanh)
                nc.vector.tensor_mul(hist[:, :, t + 1], u_sb, s_sb[:, 4:6])

            def pre1_block(b):
                for mc in range(8):
                    ps = mmps.tile([128, 512], F32, tag="mm")
                    for kc in range(2):
                        ti = kc * 8 + mc
                        nc.tensor.matmul(
                            ps[:, :BLK],
                            wih1t[:, ti * 128:(ti + 1) * 128],
                            hist0[:, kc, 1 + b * BLK:1 + (b + 1) * BLK],
                            start=(kc == 0), stop=(kc == 1))
                    nc.vector.tensor_scalar_add(
                        pre1[:, mc, b * BLK:(b + 1) * BLK], ps[:, :BLK],
                        b1[:, mc:mc + 1])

            # ---- grid micro-tasks (one ~0.7us ACT slot per LSTM pair) ----
            tasks = []
            row_state = {}

            def grid_task(j, lc):
                def run():
                    if lc == 0:
                        row_state[j] = (
                            scps.tile([1, 320], F32, tag="sc",
                                      name=f"sc0_{j}"),
                            scps.tile([1, 320], F32, tag="sc",
                                      name=f"sc1_{j}"))
                    ps0, ps1 = row_state[j]
                    hid = hidp.tile([128, N], F16, tag="hid")
                    nc.scalar.activation(hid, qT[:, lc, :], AF.Tanh,
                                         bias=pb[:, lc, j:j + 1])
                    nc.tensor.matmul(ps0, wy2c[:, lc:lc + 1],
                                     hid[:, 0:320],
                                     start=(lc == 0), stop=(lc == 4))
                    nc.tensor.matmul(ps1, wy2c[:, lc:lc + 1],
                                     hid[:, 320:640],
                                     start=(lc == 0), stop=(lc == 4))
                return run

            def grid_epilogue(j):
                def run():
                    ps0, ps1 = row_state.pop(j)
                    srow = scsb.tile([1, N], F32, tag="srow")
                    nc.vector.tensor_scalar_add(srow[:, 0:320], ps0, by2_val)
                    nc.vector.tensor_scalar_add(srow[:, 320:640], ps1,
                                                by2_val)
                    nc.sync.dma_start(d_scores[j:j + 1, :], srow)
                return run

            def emit_chunk(c, i0=0, i1=128, jj0=0, jj1=RPCH):
                # P dec part for rows [c*128+i0, c*128+i1)
                for jh in range(2):
                    ps = mmps.tile([128, 512], F32, tag="mm",
                                   name=f"pdec_{c}_{i0}_{jh}")
                    for kc in range(2):
                        nc.tensor.matmul(
                            ps[i0:i1, :320],
                            hist1[:, kc,
                                  1 + c * 128 + i0:1 + c * 128 + i1],
                            wy1ar[:, kc, jh * 320:(jh + 1) * 320],
                            start=(kc == 0), stop=(kc == 1))
                    nc.vector.tensor_add(
                        pi[i0:i1, c, jh * 320:(jh + 1) * 320],
                        pi[i0:i1, c, jh * 320:(jh + 1) * 320],
                        ps[i0:i1, :320])
                # select this core's rows jj0..jj1 of chunk c into pb
                # (one-hot columns only touch ready dec rows)
                nsel = jj1 - jj0
                for lc in range(5):
                    ps = mmps.tile([128, 512], F32, tag="mm",
                                   name=f"psel_{c}_{i0}_{lc}")
                    nc.tensor.matmul(ps[:, :nsel],
                                     pi[:, c, lc * 128:(lc + 1) * 128],
                                     onehot[:, jj0:jj1],
                                     start=True, stop=True)
                    nc.vector.tensor_copy(
                        pb[:, lc, c * RPCH + jj0:c * RPCH + jj1],
                        ps[:, :nsel])
                if os.environ.get("K_NOGRID") != "1":
                    for jj in range(jj0, jj1):
                        j = c * RPCH + jj
                        for lc in range(5):
                            tasks.append(grid_task(j, lc))
                        tasks.append(grid_epilogue(j))

            def drain_task():
                if tasks:
                    tasks.pop(0)()
                if len(tasks) > 24:
                    tasks.pop(0)()

            # ---- main pipeline ----
            for lc in range(5):
                for jh in range(2):
                    tasks.append(q_task(lc, jh))
            for c in range(NCH):
                for jh in range(2):
                    tasks.append(px_task(c, jh))
            for b in range(NBLK):
                for i in range(BLK):
                    lstm_step(0, b * BLK + i)
                    if b >= 1:
                        lstm_step(1, (b - 1) * BLK + i)
                    drain_task()
                pre1_block(b)
                if b >= 4 and b % 4 == 0:
                    emit_chunk(b // 4 - 1)
                elif b == 18:
                    emit_chunk(NCH - 1, 0, 64, 0, RPCH // 2)
            for i in range(BLK):
                lstm_step(1, (NBLK - 1) * BLK + i)
                drain_task()
            emit_chunk(NCH - 1, 64, 128, RPCH // 2, RPCH)
            while tasks:
                drain_task()

            nc.sync.dma_start(d_dec, hist1[:, :, 1:N + 1])

            # this core's dec columns (i = 8j + core), via strided dyn DMA
            pid = nc.partition_id()
            hist1_r = hist1[:, :, 1:N + 1].rearrange(
                "p h (j e) -> p h j e", e=8)
            for kc in range(2):
                nc.sync.dma_start(
                    decsel[:, kc, :],
                    hist1_r[:, kc, :, bass.ds(pid, 1)])

            main_ctx.close()

            # ---- label MLP (tail) ----
            labps = ctx.enter_context(
                tc.tile_pool(name="labps", bufs=2, space="PSUM"))
            smps = ctx.enter_context(
                tc.tile_pool(name="smps", bufs=1, space="PSUM"))
            for lc in range(5):
                ps = labps.tile([128, RPC], F32, tag="lab")
                for kc in range(8):
                    ti = kc * 5 + lc
                    nc.tensor.matmul(
                        ps, wxct[:, ti * 128:(ti + 1) * 128],
                        xxtsel[:, kc, :],
                        start=(kc == 0), stop=False)
                for kc in range(2):
                    ti = kc * 5 + lc
                    nc.tensor.matmul(
                        ps, wzdect[:, ti * 128:(ti + 1) * 128],
                        decsel[:, kc, :],
                        start=False, stop=(kc == 1))
                nc.scalar.activation(hz[:, lc, :], ps, AF.Relu,
                                     bias=bz1c[:, lc:lc + 1])

            lt_ps = smps.tile([36, RPC], F32, tag="lt")
            for lc in range(5):
                nc.tensor.matmul(lt_ps,
                                 wz2t[:, lc * 36:(lc + 1) * 36],
                                 hz[:, lc, :],
                                 start=(lc == 0), stop=(lc == 4))
            labT = consts.tile([36, RPC], F32, tag="labT")
            nc.scalar.activation(labT, lt_ps, AF.Identity,
                                 bias=bz2c[0:36, 0:1])
            nc.sync.dma_start(d_labelsT, labT)

            tr_ps = smps.tile([RPC, 36], F32, tag="tr")
            nc.tensor.transpose(tr_ps, labT, ident[0:36, 0:36])
            lab_i = consts.tile([RPC, 36], F32, tag="lab_i")
            nc.vector.tensor_copy(lab_i, tr_ps)
            mx = consts.tile([RPC, 8], F32, tag="mx")
            nc.vector.max(mx, lab_i)
            idx = consts.tile([RPC, 8], DT.uint32, tag="idx")
            nc.vector.max_index(idx, mx, lab_i)
            lidx = consts.tile([RPC, 1], DT.int32, tag="lidx")
            nc.vector.tensor_copy(lidx, idx[:, 0:1])
            nc.sync.dma_start(d_lidx, lidx)

    nc.compile()
    _split_drain_waits(nc)
    return nc


def prepare_inputs(inputs):
    x = _f(inputs["inputs"])[:, 0, :]                     # [640, 512]
    point_idx = np.asarray(inputs["point_idx"])
    istrain = int(np.asarray(inputs["istrain"]))

    whh0 = _perm_rows(_f(inputs["Whh0"]), True)
    whh1 = _perm_rows(_f(inputs["Whh1"]), True)
    wih0 = _perm_rows(_f(inputs["Wih0"]), True)
    wih1 = _perm_rows(_f(inputs["Wih1"]), True)
    b0 = _perm_rows(_f(inputs["bih0"]) + _f(inputs["bhh0"]), True)
    b1 = _perm_rows(_f(inputs["bih1"]) + _f(inputs["bhh1"]), True)

    wy1 = _f(inputs["Wy1"])
    wy1a, wy1b, wy1c = wy1[:, :H], wy1[:, H:3 * H], wy1[:, 3 * H:]
    by1 = _f(inputs["by1"])
    wy2 = _f(inputs["wy2"])
    by2_val = float(np.asarray(inputs["by2"]))

    wz1 = _f(inputs["Wz1"])
    bz1 = _f(inputs["bz1"])
    wz2 = _f(inputs["Wz2"])
    bz2 = _f(inputs["bz2"]).reshape(-1)
    if bz2.size == 1:
        bz2 = np.full(36, float(bz2[0]), np.float32)

    xg = x[point_idx]
    xx = np.concatenate([x, xg], axis=1)                  # [640, 1024]
    wxc = np.concatenate([wz1[:, :512], wz1[:, 768:1280]], axis=1)
    wzdec = wz1[:, 512:768]

    xtr = np.ascontiguousarray(x.T)                       # [512, 640]
    rep = {
        "whh0t": _pack_lhsT(whh0.T, 2, 8, np.float16),
        "whh1t": _pack_lhsT(whh1.T, 2, 8, np.float16),
        "wih1t": _pack_lhsT(wih1.T, 2, 8, np.float16),
        "wih0t": _pack_lhsT(wih0.T, 4, 8, np.float16),
        "b0": _f(b0.reshape(8, 128).T),
        "b1": _f(b1.reshape(8, 128).T),
        "xt": _f(xtr.reshape(4, 128, N)),
        "xt16": _f(xtr.reshape(4, 128, N), np.float16),
        "wy1ar": _f(wy1a.T.reshape(2, 128, N), np.float16),
        "wy1br": _f(wy1b.T.reshape(4, 128, N)),
        "wy1ct": _pack_lhsT(wy1c.T, 4, 5, np.float32),
        "by1c": _f(by1.reshape(5, 128).T),
        "wy2c": _f(wy2.reshape(5, 128).T, np.float16),
        "wxct": _pack_lhsT(wxc.T, 8, 5, np.float32),
        "wzdect": _pack_lhsT(wzdec.T, 2, 5, np.float16),
        "bz1c": _f(bz1.reshape(5, 128).T),
        "wz2t": _f(np.concatenate(
            [wz2.T[lc * 128:(lc + 1) * 128, :] for lc in range(5)], axis=1)),
        "bz2c": _f(bz2.reshape(36, 1)),
        "identf": np.eye(128, dtype=np.float32),
        "ident16": np.eye(128, dtype=np.float16),
    }
    xxt = np.ascontiguousarray(xx.T)                      # [1024, 640]
    in_maps = []
    for k in range(NCORES):
        m = dict(rep)
        oh = np.zeros((128, RPCH), np.float32)
        for jj in range(RPCH):
            oh[8 * jj + k, jj] = 1.0
        m["onehot"] = oh
        m["xxtsel"] = _f(
            np.ascontiguousarray(xxt[:, k::8]).reshape(8, 128, RPC))
        in_maps.append(m)
    return in_maps, by2_val, x, point_idx, istrain


def assemble(results, point_idx):
    scores = np.empty((N, N), np.float32)
    labels = np.empty((N, 36), np.float32)
    labels_idx = np.empty((N,), np.int32)
    for k in range(NCORES):
        scores[k::8] = results[k]["scores_part"]
        labels[k::8] = results[k]["labelsT_part"].T
        labels_idx[k::8] = results[k]["lidx_part"][:, 0]
    dec = results[0]["dec_out"]
    dec_full = np.empty((N, H), np.float32)
    dec_full[:, 0:128] = dec[:, 0, :].T.astype(np.float32)
    dec_full[:, 128:256] = dec[:, 1, :].T.astype(np.float32)
    return scores, labels, labels_idx, dec_full


def kernel(**inputs):
    in_maps, by2_val, x, point_idx, istrain = prepare_inputs(inputs)
    nc = build_program(by2_val)
    res = bass_utils.run_bass_kernel_spmd(
        nc, in_maps, core_ids=list(range(NCORES)))
    scores, labels, labels_idx, dec = assemble(res.results, point_idx)

    if istrain == 1:
        return (np.asarray(point_idx), scores, labels, labels_idx)

    # istrain != 1 fallback: p_idx = argmax(scores); recompute the (small)
    # label MLP on host from the device-computed dec.
    p_idx = np.argmax(scores, axis=1).astype(np.asarray(point_idx).dtype)
    wz1 = _f(inputs["Wz1"]); bz1 = _f(inputs["bz1"])
    wz2 = _f(inputs["Wz2"]); bz2f = _f(inputs["bz2"])
    vec = np.concatenate([x, dec, x[p_idx]], axis=1)
    labels = np.maximum(vec @ wz1.T + bz1, 0.0) @ wz2.T + bz2f
    labels_idx = np.argmax(labels, axis=1).astype(np.int32)
    return (p_idx, scores, labels, labels_idx)
